# revision 23
# baseline (speedup 1.0000x reference)
"""AFPM (adaptive per-patch modulation) kernel for 8 TRN2 NeuronCores.

Reference computation (B=8, C=64, H=W=512, K=8, HID=64):
  - d[l]: normalized distance of each 8x8 patch center from image center
  - pk[l, kk] / pb[l]: tiny MLPs of d (host-precomputable, data-independent)
  - feats[b,c,l] = sum_kk patches[b,c,kk,l] * pk[l,kk] + pb[l]
  - feats2 = conv_w @ feats + conv_b           (1x1 conv over channels)
  - out patches = patches * feats2[:, :, None, :]

Sharding: core i handles image pair (2*(i//2), 2*(i//2)+1), patch-rows
r0..r0+31 where r0 = (i%2)*32.  Each device tile packs the SAME patch-row
of the two images on the two partition halves, so one
gpsimd.partition_broadcast(channels=128) replicates the per-patch-row
weight row PKR[ph] for the whole tile (the ucode only writes from
partition 0, and compute ops need operands at equal base partitions).

Device pipeline per tile (32 tiles/core, [128, 4096] each):
  DMA x; DMA pkr row to partition 0; partition_broadcast -> PKRB
  DVE: PROD = X * PKRB;  F[(c),pw] = reduce_sum_{dy,dx}(PROD)
  PE:  G = blockdiag(conv_w.T).T @ F   (both images in one matmul)
  DVE: G' = G + PBADD[t]   (folds the pb path and conv_b)
  DVE: OUT = X * broadcast(G')  (into PROD's buffer); DMA out.
"""

import math
import sys

import numpy as np

for _p in ("/opt/trn_rl_repo",):
    if _p not in sys.path:
        sys.path.insert(0, _p)

import concourse.bass as bass
import concourse.tile as tile
from concourse import bacc, mybir
from concourse.bass_utils import run_bass_kernel_spmd

B, C, H, W, K, HID = 8, 64, 512, 512, 8, 64
NH, NW = H // K, W // K          # 64, 64
L = NH * NW                      # 4096
NT = 32                          # tiles per core (patch-rows per core)
RH = NT * K                      # 256 pixel rows per core slice
FD = K * W                       # 4096 free dim per tile half
F32 = mybir.dt.float32
BF16 = mybir.dt.bfloat16

_ERF = np.frompyfunc(math.erf, 1, 1)


def _gelu(x):
    x = np.asarray(x, np.float64)
    return 0.5 * x * (1.0 + _ERF(x / math.sqrt(2.0)).astype(np.float64))


def _host_tables(w1k, b1k, w2k, b2k, w1b, b1b, w2b, b2b, conv_w, conv_b):
    """pk/pb via the tiny MLPs; packed as PKR [NH, FD] and PBM [C, NH, NW]."""
    cy = cx = H / 2.0
    max_d = math.sqrt(cy * cy + cx * cx)
    py = np.arange(NH, dtype=np.float64) * K + K / 2.0
    px = np.arange(NW, dtype=np.float64) * K + K / 2.0
    d = np.sqrt((py - cy)[:, None] ** 2 + (px - cx)[None, :] ** 2) / max_d
    d = d.reshape(L, 1)

    pk = _gelu(d @ w1k.astype(np.float64) + b1k) @ w2k.astype(np.float64) + b2k
    pb = (_gelu(d @ w1b.astype(np.float64) + b1b) @ w2b.astype(np.float64) + b2b)[:, 0]

    # PKR[ph, dy*W + pw*K + dx] = pk[ph*NW + pw, dy*K + dx]  (bf16 on device)
    import ml_dtypes

    pkr = (
        pk.reshape(NH, NW, K, K).transpose(0, 2, 1, 3).reshape(NH, FD)
    ).astype(ml_dtypes.bfloat16)

    # G' = conv_w @ F + (cw1 * pb + conv_b)
    cw1 = conv_w.astype(np.float64).sum(axis=1)
    pbm = (cw1[:, None] * pb[None, :] + conv_b.astype(np.float64)[:, None]).reshape(
        C, NH, NW
    )

    bd = np.zeros((128, 128), np.float32)
    bd[0:C, 0:C] = conv_w.T
    bd[C:128, C:128] = conv_w.T
    return pkr, pbm, bd


def build_program():
    nc = bacc.Bacc("TRN2", target_bir_lowering=False, debug=False, num_devices=8)
    x_d = nc.dram_tensor("x", [2, C, RH, W], BF16, kind="ExternalInput")
    pkr_d = nc.dram_tensor("pkr", [NT, FD], BF16, kind="ExternalInput")
    pbadd_d = nc.dram_tensor("pbadd", [128, NT * NW], F32, kind="ExternalInput")
    bd_d = nc.dram_tensor("bd", [128, 128], F32, kind="ExternalInput")
    out_d = nc.dram_tensor("out", [2, C, RH, W], BF16, kind="ExternalOutput")

    # [t, (u c)=128, dy, w] views of the DRAM image slices
    xr = x_d.ap().rearrange("u c (t dy) w -> t (u c) dy w", dy=K)
    outr = out_d.ap().rearrange("u c (t dy) w -> t (u c) dy w", dy=K)

    with tile.TileContext(nc) as tc:
        with (
            tc.tile_pool(name="const", bufs=1) as constp,
            tc.tile_pool(name="xbp", bufs=4) as xbp,
            tc.tile_pool(name="prodp", bufs=3) as prodp,
            tc.tile_pool(name="php", bufs=2) as php,
            tc.tile_pool(name="pkrbp", bufs=3) as pkrbp,
            tc.tile_pool(name="stgp", bufs=3) as stgp,
            tc.tile_pool(name="smallp", bufs=4) as smallp,
            tc.tile_pool(name="psump", bufs=4, space="PSUM") as psump,
        ):
            pbadd = constp.tile([128, NT * NW], F32)
            nc.sync.dma_start(pbadd[:], pbadd_d[:])
            bdt = constp.tile([128, 128], F32)
            nc.sync.dma_start(bdt[:], bd_d[:])

            for t in range(NT):
                xb = xbp.tile([128, FD], BF16)
                nc.sync.dma_start(
                    xb.rearrange("p (dy w) -> p dy w", dy=K), xr[t]
                )

                stg = stgp.tile([1, FD], BF16)
                nc.sync.dma_start(stg[:], pkr_d[t : t + 1, :])

                pkrb = pkrbp.tile([128, FD], BF16)
                nc.gpsimd.partition_broadcast(pkrb[:], stg[0:1, :], channels=128)

                # product as two half tiles (dy 0:4 / dy 4:8): the fold-add
                # then reads two distinct SBUF tensors (dual-port friendly)
                h = FD // 2
                plo = php.tile([128, h], BF16, tag="plo")
                phi = php.tile([128, h], BF16, tag="phi")
                nc.vector.tensor_mul(plo[:], xb[:, 0:h], pkrb[:, 0:h])
                nc.vector.tensor_mul(phi[:], xb[:, h:FD], pkrb[:, h:FD])
                a1 = php.tile([128, h], BF16, tag="a1")
                nc.vector.tensor_tensor(
                    a1[:], plo[:], phi[:], op=mybir.AluOpType.add
                )
                f = smallp.tile([128, NW], F32)
                nc.vector.tensor_reduce(
                    f[:],
                    a1.rearrange("p (dy pw dx) -> p pw dy dx", dy=K // 2, pw=NW),
                    axis=mybir.AxisListType.XY,
                    op=mybir.AluOpType.add,
                )

                g = psump.tile([128, NW], F32)
                nc.tensor.matmul(g[:], bdt[:], f[:])

                gs = smallp.tile([128, NW], F32)
                nc.vector.tensor_tensor(
                    gs[:], g[:], pbadd[:, t * NW : (t + 1) * NW],
                    op=mybir.AluOpType.add,
                )
                # cast G' to bf16 expanded over dx (one ACT op, so the
                # modulation mul below sees a dense 512-elem inner run)
                gexp = smallp.tile([128, NW * K], BF16, tag="gexp")
                ge3 = gexp.rearrange("p (pw dx) -> p pw dx", dx=K)
                gs3 = gs.rearrange("p (pw a) -> p pw a", a=1)
                ge3b, gs3b = bass.broadcast_tensor_aps(ge3, gs3)
                nc.scalar.copy(ge3b, gs3b)

                # OUT = Xb * broadcast(GEXP) over dy, bf16
                prod = prodp.tile([128, FD], BF16)
                o3 = prod.rearrange("p (dy q) -> p dy q", dy=K)
                x3 = xb.rearrange("p (dy q) -> p dy q", dy=K)
                g3 = gexp.rearrange("p (a q) -> p a q", a=1)
                x3b, g3b = bass.broadcast_tensor_aps(x3, g3)
                nc.vector.tensor_tensor(o3, x3b, g3b, op=mybir.AluOpType.mult)

                # DMA the bf16 result out on the scalar HWDGE ring
                # (separate FIFO from the input ring); host upcasts to f32
                nc.scalar.dma_start(
                    outr[t], prod.rearrange("p (dy w) -> p dy w", dy=K)
                )

    nc.compile()
    return nc


_PROGRAM = None
LAST_RESULT = None


def make_in_maps(x, pkr, pbm, bd):
    in_maps = []
    for i in range(8):
        pair, half = i // 2, i % 2
        r0 = half * NT
        import ml_dtypes

        x_core = np.ascontiguousarray(
            x[2 * pair : 2 * pair + 2, :, r0 * K : (r0 + NT) * K, :]
        ).astype(ml_dtypes.bfloat16)
        pkr_core = np.ascontiguousarray(pkr[r0 : r0 + NT])
        sub = pbm[:, r0 : r0 + NT, :].reshape(C, NT * NW)
        pbadd_core = np.concatenate([sub, sub], axis=0).astype(np.float32)
        in_maps.append(
            {"x": x_core, "pkr": pkr_core, "pbadd": pbadd_core, "bd": bd}
        )
    return in_maps


def kernel(**inputs):
    global _PROGRAM, LAST_RESULT
    x = np.ascontiguousarray(np.asarray(inputs["x"], dtype=np.float32))
    pkr, pbm, bd = _host_tables(
        *[
            np.asarray(inputs[k], dtype=np.float32)
            for k in (
                "w1k", "b1k", "w2k", "b2k",
                "w1b", "b1b", "w2b", "b2b",
                "conv_w", "conv_b",
            )
        ]
    )
    if _PROGRAM is None:
        _PROGRAM = build_program()
    nc = _PROGRAM

    in_maps = make_in_maps(x, pkr, pbm, bd)
    res = run_bass_kernel_spmd(nc, in_maps, list(range(8)))
    LAST_RESULT = res

    out = np.empty((B, C, H, W), np.float32)
    for i in range(8):
        pair, half = i // 2, i % 2
        r0 = half * NT
        out[2 * pair : 2 * pair + 2, :, r0 * K : (r0 + NT) * K, :] = res.results[
            i
        ]["out"].astype(np.float32)
    return out
